# revision 2
# baseline (speedup 1.0000x reference)
"""Trainium2 Bass kernel for nn_DirModel (quaternion Dirac GNN message passing).

Strategy (8 NeuronCores, B=2 samples):
  - 4 cores per sample: core c owns sample s=c//4 and slice r=c%4 of the face
    rows (Di output) / node rows (DiA output).
  - The big incidence operators Di [2,8192,4096] / DiA [2,4096,8192] are
    host-side transposed/permuted/sliced, cast to fp8(e4m3), and kept RESIDENT
    in SBUF (8 MB + 8 MB per core) across all 5 blocks: no per-block HBM
    streaming of the big matrices.
  - Big matmuls keep the activations stationary (lhsT [128,16] quaternion
    slices) and stream the resident fp8 matrix as moving operand (N=512),
    accumulating over the contraction dim in PSUM.
  - Node/face state (vT/fT: channels x rows, bf16) is replicated on every
    core; per block each core computes its output slice and slices are
    exchanged with one 8-rank AllGather per direction (64KB / 32KB per rank).
  - The SPMD program is identical on all cores; sample selection is
    data-driven: per-core 0/1 masks multiply into the BN affine scale/shift so
    the non-owned sample's stationary operand is exactly zero and contributes
    nothing to the shared PSUM accumulation.
  - BatchNorm batch stats (over both samples) are computed locally from the
    replicated state. elu(x) = relu(x) + min(exp(x)-1, 0) in 3 ops with the
    stats sums piggybacked on accum_out.
"""

import numpy as np
import ml_dtypes

import concourse.bass as bass
import concourse.mybir as mybir
import concourse.tile as tile
from concourse import bacc
from concourse.bass_utils import run_bass_kernel_spmd

B, N, F, C = 2, 1024, 2048, 64
NB = 5
EPS = 1e-5
NCORES = 8

F32 = mybir.dt.float32
BF16 = mybir.dt.bfloat16
FP8 = mybir.dt.float8e4
NP_BF16 = ml_dtypes.bfloat16
NP_FP8 = ml_dtypes.float8_e4m3
AF = mybir.ActivationFunctionType
ALU = mybir.AluOpType
RG = [list(range(NCORES))]


def _build():
    nc = bacc.Bacc(
        "TRN2",
        target_bir_lowering=False,
        debug=False,
        enable_asserts=False,
        num_devices=NCORES,
    )

    # ---------------- DRAM I/O ----------------
    dit_d = nc.dram_tensor("dit", [128, 32, 2048], FP8, kind="ExternalInput")
    diat_d = nc.dram_tensor("diat", [128, 64, 1024], FP8, kind="ExternalInput")
    inpT_d = nc.dram_tensor("inpT", [3, B, N], BF16, kind="ExternalInput")
    w_in_d = nc.dram_tensor("w_in", [3, C], BF16, kind="ExternalInput")
    b_in_d = nc.dram_tensor("b_in", [C, 1], F32, kind="ExternalInput")
    w0_d = nc.dram_tensor("w0", [C, NB, C], FP8, kind="ExternalInput")
    w1_d = nc.dram_tensor("w1", [C, NB, C], FP8, kind="ExternalInput")
    b0_d = nc.dram_tensor("b0", [1, NB, C], BF16, kind="ExternalInput")
    b1_d = nc.dram_tensor("b1", [1, NB, C], BF16, kind="ExternalInput")
    g0_d = nc.dram_tensor("g0", [C, NB], F32, kind="ExternalInput")
    be0_d = nc.dram_tensor("be0", [C, NB], F32, kind="ExternalInput")
    g1_d = nc.dram_tensor("g1", [C, NB], F32, kind="ExternalInput")
    be1_d = nc.dram_tensor("be1", [C, NB], F32, kind="ExternalInput")
    msel_d = nc.dram_tensor("msel", [C, B], F32, kind="ExternalInput")
    bn2g_d = nc.dram_tensor("bn2g", [C, 1], F32, kind="ExternalInput")
    bn2b_d = nc.dram_tensor("bn2b", [C, 1], F32, kind="ExternalInput")
    w2_d = nc.dram_tensor("w2", [C, C], BF16, kind="ExternalInput")
    b2_d = nc.dram_tensor("b2", [1, C], BF16, kind="ExternalInput")
    maskc_d = nc.dram_tensor("maskc", [128, 8, B], BF16, kind="ExternalInput")
    maskrow_d = nc.dram_tensor("maskrow", [B, N], BF16, kind="ExternalInput")
    wfc_d = nc.dram_tensor("wfc", [C, 10], BF16, kind="ExternalInput")
    bfc_d = nc.dram_tensor("bfc", [B, 10], F32, kind="ExternalInput")
    out_d = nc.dram_tensor("out", [B, 10], F32, kind="ExternalOutput")

    with tile.TileContext(nc) as tc:
        with (
            tc.tile_pool(name="res", bufs=1) as res,
            tc.tile_pool(name="sb", bufs=2) as sb,
            tc.tile_pool(name="sc", bufs=2) as sc,
            tc.tile_pool(name="st", bufs=4) as st,
            tc.tile_pool(name="pacc", bufs=1, space="PSUM") as pacc,
            tc.tile_pool(name="px", bufs=2, space="PSUM") as px,
            tc.tile_pool(name="pm", bufs=1, space="PSUM") as pm,
            tc.tile_pool(name="dram", bufs=2, space="DRAM") as dram,
        ):
            # ------------- resident loads -------------
            dit_sb = res.tile([128, 32, 2048], FP8)
            diat_sb = res.tile([128, 64, 1024], FP8)
            nc.sync.dma_start(dit_sb[:], dit_d.ap())
            nc.sync.dma_start(diat_sb[:], diat_d.ap())

            def load(name, shape, dtype, src):
                t = res.tile(shape, dtype, name=name)
                nc.sync.dma_start(t[:], src.ap())
                return t

            w_in_sb = load("w_in_sb", [3, C], BF16, w_in_d)
            b_in_sb = load("b_in_sb", [C, 1], F32, b_in_d)
            w0_sb = load("w0_sb", [C, NB, C], FP8, w0_d)
            w1_sb = load("w1_sb", [C, NB, C], FP8, w1_d)
            b0_sb = load("b0_sb", [1, NB, C], BF16, b0_d)
            b1_sb = load("b1_sb", [1, NB, C], BF16, b1_d)
            g0_sb = load("g0_sb", [C, NB], F32, g0_d)
            be0_sb = load("be0_sb", [C, NB], F32, be0_d)
            g1_sb = load("g1_sb", [C, NB], F32, g1_d)
            be1_sb = load("be1_sb", [C, NB], F32, be1_d)
            msel_sb = load("msel_sb", [C, B], F32, msel_d)
            bn2g_sb = load("bn2g_sb", [C, 1], F32, bn2g_d)
            bn2b_sb = load("bn2b_sb", [C, 1], F32, bn2b_d)
            w2_sb = load("w2_sb", [C, C], BF16, w2_d)
            b2_sb = load("b2_sb", [1, C], BF16, b2_d)
            maskc_sb = load("maskc_sb", [128, 8, B], BF16, maskc_d)
            maskrow_sb = load("maskrow_sb", [B, N], BF16, maskrow_d)
            wfc_sb = load("wfc_sb", [C, 10], BF16, wfc_d)
            bfc_sb = load("bfc_sb", [B, 10], F32, bfc_d)
            inpT_sb = load("inpT_sb", [3, B, N], BF16, inpT_d)

            ones_bf = res.tile([1, 128], BF16)
            nc.vector.memset(ones_bf[:], 1.0)

            # ------------- state -------------
            vT = [res.tile([C, N], BF16, name=f"vT{s}") for s in range(B)]
            fT = [res.tile([C, F], BF16, name=f"fT{s}") for s in range(B)]

            # initial v = inputs @ W_in + b_in   (vT: channels x nodes)
            for s in range(B):
                for h in range(2):
                    ps0 = pm.tile([C, 512], F32, tag="misc", name=f"psi{s}{h}")
                    nc.tensor.matmul(
                        ps0[:],
                        w_in_sb[:],
                        inpT_sb[:, s, 512 * h:512 * (h + 1)],
                        start=True,
                        stop=True,
                    )
                    nc.vector.tensor_scalar(
                        vT[s][:, 512 * h:512 * (h + 1)], ps0[:],
                        b_in_sb[:].opt(), None, ALU.add,
                    )
                nc.vector.memset(fT[s][:], 0.0)

            def elu_stats(src, P, R, nm):
                """elu(src) -> ev [P,R] bf16, plus (sum, sumsq) over free dim.

                elu(x) = max(x,0) + min(exp(x)-1, 0)
                """
                e = sc.tile([P, R], BF16, tag="eT", name=f"e{nm}")
                nc.scalar.activation(e[:], src, AF.Exp)
                nc.vector.tensor_scalar(e[:], e[:], -1.0, 0.0, ALU.add, ALU.min)
                ev = sc.tile([P, R], BF16, tag="evT", name=f"ev{nm}")
                ssum = st.tile([P, 1], F32, tag="ssum", name=f"ssum{nm}")
                nc.vector.scalar_tensor_tensor(
                    ev[:], src, 0.0, e[:], ALU.max, ALU.add, accum_out=ssum[:]
                )
                ssq = st.tile([P, 1], F32, tag="ssq", name=f"ssq{nm}")
                sqd = sc.tile([P, R], BF16, tag="eT", name=f"sq{nm}")
                nc.scalar.activation(sqd[:], ev[:], AF.Square, accum_out=ssq[:])
                return ev, ssum, ssq

            def bn_sel_scales(ssums, ssqs, g_ap, be_ap, T, nm):
                """Global BN stats -> per-sample (scale_s, shneg_s) [C,1] f32,
                pre-multiplied by this core's sample-selector mask."""
                msum = st.tile([C, 1], F32, tag="bns", name=f"ms{nm}")
                nc.vector.tensor_add(msum[:], ssums[0][:], ssums[1][:])
                mean = st.tile([C, 1], F32, tag="bns", name=f"mn{nm}")
                nc.vector.tensor_scalar_mul(mean[:], msum[:], 1.0 / T)
                qsum = st.tile([C, 1], F32, tag="bns", name=f"qs{nm}")
                nc.vector.tensor_add(qsum[:], ssqs[0][:], ssqs[1][:])
                m2 = st.tile([C, 1], F32, tag="bns", name=f"m2{nm}")
                nc.vector.tensor_mul(m2[:], mean[:], mean[:])
                varp = st.tile([C, 1], F32, tag="bns", name=f"vp{nm}")
                nc.vector.scalar_tensor_tensor(
                    varp[:], qsum[:], 1.0 / T, m2[:], ALU.mult, ALU.subtract
                )
                nc.vector.tensor_scalar_add(varp[:], varp[:], EPS)
                sd = st.tile([C, 1], F32, tag="bns", name=f"sd{nm}")
                nc.scalar.activation(sd[:], varp[:], AF.Sqrt)
                rstd = st.tile([C, 1], F32, tag="bns", name=f"rstd{nm}")
                nc.vector.reciprocal(rstd[:], sd[:])
                scale = st.tile([C, 1], F32, tag="bns", name=f"scale{nm}")
                nc.vector.tensor_mul(scale[:], rstd[:], g_ap)
                shneg = st.tile([C, 1], F32, tag="bns", name=f"shneg{nm}")
                # shneg = mean*scale - be ; affine is x*scale - shneg
                nc.vector.scalar_tensor_tensor(
                    shneg[:], mean[:], scale[:].opt(), be_ap, ALU.mult, ALU.subtract
                )
                outs = []
                for s in range(B):
                    sc_s = st.tile([C, 1], F32, tag="selsc", name=f"scs{nm}{s}")
                    nc.vector.tensor_mul(sc_s[:], scale[:], msel_sb[:, s:s + 1])
                    sh_s = st.tile([C, 1], F32, tag="selsh", name=f"shs{nm}{s}")
                    nc.vector.tensor_mul(sh_s[:], shneg[:], msel_sb[:, s:s + 1])
                    outs.append((sc_s, sh_s))
                return outs

            def sel_affine(evs, sels, R, dtype, nm):
                """stuff_s = ev_s*scale_s - shneg_s (zero for non-owned sample)"""
                stuffs = []
                for s in range(B):
                    t = sc.tile([C, R], dtype, tag="stuff", name=f"stuff{nm}{s}")
                    nc.vector.tensor_scalar(
                        t[:], evs[s][:], sels[s][0][:].opt(), sels[s][1][:].opt(),
                        ALU.mult, ALU.subtract,
                    )
                    stuffs.append(t)
                return stuffs

            def produce_rows(stuffs, nchunks, w_ap, b_ap, out_dtype, nm):
                """rows[t][128,C] = elu(sum_s stuffs[s][:,128t:].T @ W + b)"""
                rows = []
                for t in range(nchunks):
                    psx = px.tile([128, C], F32, tag="psx", name=f"psx{nm}{t}")
                    nc.tensor.matmul(
                        psx[:], stuffs[0][:, 128 * t:128 * (t + 1)], w_ap,
                        start=True, stop=False,
                    )
                    nc.tensor.matmul(
                        psx[:], stuffs[1][:, 128 * t:128 * (t + 1)], w_ap,
                        start=False, stop=False,
                    )
                    nc.tensor.matmul(
                        psx[:], ones_bf[:], b_ap, start=False, stop=True,
                    )
                    e2 = sc.tile([128, C], BF16, tag="e2", name=f"e2{nm}{t}")
                    nc.scalar.activation(e2[:], psx[:], AF.Exp)
                    nc.vector.tensor_scalar(
                        e2[:], e2[:], -1.0, 0.0, ALU.add, ALU.min
                    )
                    row = sc.tile([128, C], out_dtype, tag=f"rowt{t}",
                                  name=f"row{nm}{t}")
                    nc.vector.scalar_tensor_tensor(
                        row[:], psx[:], 0.0, e2[:], ALU.max, ALU.add
                    )
                    rows.append(row)
                return rows

            for i in range(NB):
                # ======== v side ========
                ev0, ss0, sq0 = elu_stats(vT[0][:], C, N, f"v0_{i}")
                ev1, ss1, sq1 = elu_stats(vT[1][:], C, N, f"v1_{i}")
                sels = bn_sel_scales(
                    (ss0, ss1), (sq0, sq1),
                    g0_sb[:, i:i + 1].opt(), be0_sb[:, i:i + 1].opt(),
                    float(B * N), f"v{i}",
                )
                stuffs = sel_affine((ev0, ev1), sels, N, FP8, f"v{i}")
                xrs = produce_rows(
                    stuffs, 8, w0_sb[:, i, :], b0_sb[:, i, :], FP8, f"x{i}"
                )

                # Di matmul: psf[j] [16,512] accumulates over contraction kk
                psf = [pacc.tile([16, 512], F32, tag=f"pf{j}", name=f"pf{i}_{j}")
                       for j in range(4)]
                for kk in range(32):
                    n8, jj = kk // 4, kk % 4
                    for j in range(4):
                        nc.tensor.matmul(
                            psf[j][:],
                            xrs[n8][:, 16 * jj:16 * (jj + 1)],
                            dit_sb[:, kk, 512 * j:512 * (j + 1)],
                            start=(kk == 0), stop=(kk == 31),
                        )
                # stage + AllGather f_new
                stgf = sb.tile([16, 4, 512], BF16, tag="stgf", name=f"stgf{i}")
                for j in range(4):
                    nc.vector.tensor_copy(stgf[:, j, :], psf[j][:])
                aginf = dram.tile([C, 512], BF16, tag="aginf", name=f"aginf{i}")
                nc.sync.dma_start(
                    aginf[:].rearrange("(j q) n -> q j n", j=4), stgf[:]
                )
                agoutf = dram.tile([NCORES * C, 512], BF16, tag="agoutf",
                                   name=f"agoutf{i}", addr_space="Shared")
                nc.gpsimd.collective_compute(
                    "AllGather", ALU.bypass, replica_groups=RG,
                    ins=[aginf.opt()], outs=[agoutf.opt()],
                )
                agfv = agoutf[:].rearrange("(s r c) n -> s c r n", s=2, r=4)
                for s in range(B):
                    gf = sb.tile([C, 4, 512], BF16, tag="gf", name=f"gf{i}_{s}")
                    nc.sync.dma_start(gf[:], agfv[s])
                    nc.vector.tensor_tensor(
                        fT[s][:], fT[s][:],
                        gf[:].rearrange("c r n -> c (r n)"), ALU.add,
                    )

                # ======== f side ========
                ef0, fs0, fq0 = elu_stats(fT[0][:], C, F, f"f0_{i}")
                ef1, fs1, fq1 = elu_stats(fT[1][:], C, F, f"f1_{i}")
                fsels = bn_sel_scales(
                    (fs0, fs1), (fq0, fq1),
                    g1_sb[:, i:i + 1].opt(), be1_sb[:, i:i + 1].opt(),
                    float(B * F), f"f{i}",
                )
                fstuffs = sel_affine((ef0, ef1), fsels, F, FP8, f"f{i}")
                yrs = produce_rows(
                    fstuffs, 16, w1_sb[:, i, :], b1_sb[:, i, :], FP8, f"y{i}"
                )

                psv = [pacc.tile([16, 512], F32, tag=f"pf{nt}", name=f"pv{i}_{nt}")
                       for nt in range(2)]
                for kk in range(64):
                    pc, jj = kk // 4, kk % 4
                    for nt in range(2):
                        nc.tensor.matmul(
                            psv[nt][:],
                            yrs[pc][:, 16 * jj:16 * (jj + 1)],
                            diat_sb[:, kk, 512 * nt:512 * (nt + 1)],
                            start=(kk == 0), stop=(kk == 63),
                        )
                stgv = sb.tile([16, 4, 256], BF16, tag="stgv", name=f"stgv{i}")
                for nt in range(2):
                    nc.vector.tensor_copy(stgv[:, 2 * nt, :], psv[nt][:, 0:256])
                    nc.vector.tensor_copy(stgv[:, 2 * nt + 1, :], psv[nt][:, 256:512])
                aginv = dram.tile([C, 256], BF16, tag="aginv", name=f"aginv{i}")
                nc.sync.dma_start(
                    aginv[:].rearrange("(j q) n -> q j n", j=4), stgv[:]
                )
                agoutv = dram.tile([NCORES * C, 256], BF16, tag="agoutv",
                                   name=f"agoutv{i}", addr_space="Shared")
                nc.gpsimd.collective_compute(
                    "AllGather", ALU.bypass, replica_groups=RG,
                    ins=[aginv.opt()], outs=[agoutv.opt()],
                )
                agvv = agoutv[:].rearrange("(s r c) n -> s c r n", s=2, r=4)
                for s in range(B):
                    gv = sb.tile([C, 4, 256], BF16, tag="gv", name=f"gv{i}_{s}")
                    nc.sync.dma_start(gv[:], agvv[s])
                    nc.vector.tensor_tensor(
                        vT[s][:], vT[s][:],
                        gv[:].rearrange("c r n -> c (r n)"), ALU.add,
                    )

            # ======== head (computed fully for both samples on every core) ===
            hv0, hs0, hq0 = elu_stats(vT[0][:], C, N, "h0")
            hv1, hs1, hq1 = elu_stats(vT[1][:], C, N, "h1")
            # no sample selection in the head: use raw scale/shift for both
            msumh = st.tile([C, 1], F32, tag="bns", name="msh")
            nc.vector.tensor_add(msumh[:], hs0[:], hs1[:])
            meanh = st.tile([C, 1], F32, tag="bns", name="mnh")
            nc.vector.tensor_scalar_mul(meanh[:], msumh[:], 1.0 / (B * N))
            qsumh = st.tile([C, 1], F32, tag="bns", name="qsh")
            nc.vector.tensor_add(qsumh[:], hq0[:], hq1[:])
            m2h = st.tile([C, 1], F32, tag="bns", name="m2h")
            nc.vector.tensor_mul(m2h[:], meanh[:], meanh[:])
            varph = st.tile([C, 1], F32, tag="bns", name="vph")
            nc.vector.scalar_tensor_tensor(
                varph[:], qsumh[:], 1.0 / (B * N), m2h[:], ALU.mult, ALU.subtract
            )
            nc.vector.tensor_scalar_add(varph[:], varph[:], EPS)
            sdh = st.tile([C, 1], F32, tag="bns", name="sdh")
            nc.scalar.activation(sdh[:], varph[:], AF.Sqrt)
            rstdh = st.tile([C, 1], F32, tag="bns", name="rstdh")
            nc.vector.reciprocal(rstdh[:], sdh[:])
            scaleh = st.tile([C, 1], F32, tag="bns", name="scaleh")
            nc.vector.tensor_mul(scaleh[:], rstdh[:], bn2g_sb[:].opt())
            shnegh = st.tile([C, 1], F32, tag="bns", name="shnegh")
            nc.vector.scalar_tensor_tensor(
                shnegh[:], meanh[:], scaleh[:].opt(), bn2b_sb[:].opt(),
                ALU.mult, ALU.subtract,
            )
            hev = [hv0, hv1]
            pooled = sb.tile([C, B], BF16, tag="pooled")
            for s in range(B):
                stuff2 = sc.tile([C, N], BF16, tag="stuff", name=f"stuff2_{s}")
                nc.vector.tensor_scalar(
                    stuff2[:], hev[s][:], scaleh[:].opt(), shnegh[:].opt(),
                    ALU.mult, ALU.subtract,
                )
                v2r = []
                for t in range(8):
                    psx = px.tile([128, C], F32, tag="psx", name=f"psh{s}{t}")
                    nc.tensor.matmul(
                        psx[:], stuff2[:, 128 * t:128 * (t + 1)], w2_sb[:],
                        start=True, stop=False,
                    )
                    nc.tensor.matmul(
                        psx[:], ones_bf[:], b2_sb[:], start=False, stop=True,
                    )
                    e2 = sc.tile([128, C], BF16, tag="e2", name=f"e2h{s}{t}")
                    nc.scalar.activation(e2[:], psx[:], AF.Exp)
                    nc.vector.tensor_scalar(
                        e2[:], e2[:], -1.0, 0.0, ALU.add, ALU.min
                    )
                    row = sc.tile([128, C], BF16, tag=f"rowt{t}", name=f"rh{s}{t}")
                    nc.vector.scalar_tensor_tensor(
                        row[:], psx[:], 0.0, e2[:], ALU.max, ALU.add
                    )
                    v2r.append(row)
                pp = pm.tile([C, 1], F32, tag="misc", name=f"pp{s}")
                for t in range(8):
                    nc.tensor.matmul(
                        pp[:], v2r[t][:], maskc_sb[:, t, s:s + 1],
                        start=(t == 0), stop=(t == 7),
                    )
                nc.vector.tensor_copy(pooled[:, s:s + 1], pp[:])
            msum = st.tile([B, 1], F32, tag="hd", name="msum")
            nc.vector.tensor_reduce(
                msum[:], maskrow_sb[:], mybir.AxisListType.X, ALU.add
            )
            rec = st.tile([B, 1], F32, tag="hd", name="rec")
            nc.vector.reciprocal(rec[:], msum[:])
            pl = pm.tile([B, 10], F32, tag="misc", name="pl")
            nc.tensor.matmul(pl[:], pooled[:], wfc_sb[:], start=True, stop=True)
            lu = sb.tile([B, 10], F32, tag="hd2", name="lu")
            nc.vector.scalar_tensor_tensor(
                lu[:], pl[:], rec[:].opt(), bfc_sb[:], ALU.mult, ALU.add
            )
            rmax = st.tile([B, 1], F32, tag="hd", name="rmax")
            nc.vector.tensor_reduce(rmax[:], lu[:], mybir.AxisListType.X, ALU.max)
            t2 = sb.tile([B, 10], F32, tag="hd2", name="t2")
            nc.vector.tensor_scalar(t2[:], lu[:], rmax[:].opt(), None, ALU.subtract)
            et = sb.tile([B, 10], F32, tag="hd2", name="et")
            se = st.tile([B, 1], F32, tag="hd", name="se")
            nc.scalar.activation(et[:], t2[:], AF.Exp, accum_out=se[:])
            ls = st.tile([B, 1], F32, tag="hd", name="ls")
            nc.scalar.activation(ls[:], se[:], AF.Ln)
            outv = sb.tile([B, 10], F32, tag="hd2", name="outv")
            nc.vector.tensor_scalar(outv[:], t2[:], ls[:].opt(), None, ALU.subtract)
            nc.sync.dma_start(out_d.ap(), outv[:])

    nc.compile()
    return nc


_NC = None


def _get_nc():
    global _NC
    if _NC is None:
        _NC = _build()
    return _NC


def _host_prep(inputs):
    """Build the 8 per-core input maps. Core c: sample s=c//4, slice r=c%4."""
    Di = np.ascontiguousarray(np.asarray(inputs["Di"]), np.float32)
    DiA = np.ascontiguousarray(np.asarray(inputs["DiA"]), np.float32)
    inp = np.asarray(inputs["inputs"], np.float32)
    mask = np.asarray(inputs["mask"], np.float32)[:, :, 0]   # [2, 1024]

    base = {}
    base["w_in"] = np.asarray(inputs["W_in"]).astype(NP_BF16)
    base["b_in"] = np.asarray(inputs["b_in"]).astype(np.float32).reshape(C, 1)
    base["w0"] = np.ascontiguousarray(
        np.asarray(inputs["rn_W0"]).transpose(1, 0, 2)).astype(NP_FP8)
    base["w1"] = np.ascontiguousarray(
        np.asarray(inputs["rn_W1"]).transpose(1, 0, 2)).astype(NP_FP8)
    base["b0"] = np.asarray(inputs["rn_b0"]).astype(NP_BF16)[None, :, :]
    base["b1"] = np.asarray(inputs["rn_b1"]).astype(NP_BF16)[None, :, :]
    base["g0"] = np.ascontiguousarray(
        np.asarray(inputs["rn_g0"]).T).astype(np.float32)
    base["be0"] = np.ascontiguousarray(
        np.asarray(inputs["rn_be0"]).T).astype(np.float32)
    base["g1"] = np.ascontiguousarray(
        np.asarray(inputs["rn_g1"]).T).astype(np.float32)
    base["be1"] = np.ascontiguousarray(
        np.asarray(inputs["rn_be1"]).T).astype(np.float32)
    base["bn2g"] = np.asarray(inputs["bn2_g"]).astype(np.float32).reshape(C, 1)
    base["bn2b"] = np.asarray(inputs["bn2_b"]).astype(np.float32).reshape(C, 1)
    base["w2"] = np.asarray(inputs["W2"]).astype(NP_BF16)
    base["b2"] = np.asarray(inputs["b2"]).astype(NP_BF16).reshape(1, C)
    base["wfc"] = np.asarray(inputs["Wfc"]).astype(NP_BF16)
    base["bfc"] = np.broadcast_to(
        np.asarray(inputs["bfc"], np.float32), (B, 10)).copy()
    base["inpT"] = np.ascontiguousarray(inp.transpose(2, 0, 1)).astype(NP_BF16)
    base["maskc"] = np.ascontiguousarray(
        mask.reshape(2, 8, 128).transpose(2, 1, 0)).astype(NP_BF16)
    base["maskrow"] = mask.astype(NP_BF16)

    in_maps = []
    for c in range(NCORES):
        s, r = c // 4, c % 4
        m = dict(base)
        Dr = Di[s].reshape(F, 4, N, 4)          # [p, j, n, jj]
        P4 = Dr[512 * r:512 * (r + 1)]          # [512, 4, 1024, 4]
        DiTg = P4.reshape(512, 4, 8, 128, 4).transpose(2, 4, 3, 1, 0) \
                 .reshape(4096, 2048)           # rows (n8,jj,n'), cols (j,p')
        m["dit"] = np.ascontiguousarray(
            DiTg.reshape(32, 128, 2048).transpose(1, 0, 2)).astype(NP_FP8)
        A = DiA[s].reshape(N, 4, F, 4)          # [n, j, p, jj]
        A4 = A[256 * r:256 * (r + 1)]           # [256, 4, 2048, 4]
        DiATg = A4.reshape(256, 4, 16, 128, 4).transpose(2, 4, 3, 1, 0) \
                  .reshape(8192, 1024)          # rows (pc,jj,p''), cols (j,n')
        m["diat"] = np.ascontiguousarray(
            DiATg.reshape(64, 128, 1024).transpose(1, 0, 2)).astype(NP_FP8)
        msel = np.zeros((C, B), np.float32)
        msel[:, s] = 1.0
        m["msel"] = msel
        in_maps.append(m)
    return in_maps


def _run(inputs, trace=False, **kw):
    nc = _get_nc()
    in_maps = _host_prep(inputs)
    res = run_bass_kernel_spmd(
        nc, in_maps, core_ids=list(range(NCORES)), trace=trace, **kw
    )
    out = np.asarray(res.results[0]["out"], np.float32).copy()
    return out, res


def kernel(**inputs):
    out, _ = _run(inputs, trace=False)
    return out


# revision 3
# speedup vs baseline: 1.3481x; 1.3481x over previous
"""Trainium2 Bass kernel for nn_DirModel (quaternion Dirac GNN message passing).

Strategy (8 NeuronCores, B=2 samples):
  - 4 cores per sample: core c owns sample s=c//4 and slice r=c%4 of the face
    rows (Di output) / node rows (DiA output).
  - Di/DiA are host-side transposed/permuted/sliced, cast to fp8(e4m3), and
    kept RESIDENT in SBUF (8+8 MB per core) across all 5 blocks.
  - Big matmuls keep the activations stationary (lhsT [128,16] quaternion
    slices) and stream the resident fp8 matrix as moving operand (N=512),
    accumulating over the contraction in PSUM. The 4 (Di) / 2 (DiA) output
    groups of one contraction chunk run column-tiled (tile_position=(0,32j))
    so their LDWEIGHTS+MATMUL pairs overlap in the PE array.
  - Both samples are stacked on the partition axis: states vTB/fTB are
    [128, rows] bf16 with partitions 64*s+c. All elementwise/BN work runs at
    full 128-lane width; the per-node linears contract K=128 (both samples at
    once) with duplicated weights, the non-owned sample zeroed via a per-core
    selector mask folded into the BN affine (SPMD program stays identical).
  - Per block, slices are exchanged with one 8-rank AllGather per direction.
  - BN batch stats come free: elu(x)=max(x,0)+min(exp(x)-1,0) with sum/sumsq
    piggybacked on accum_out; cross-partition stat combines use two tiny
    SBUF-to-SBUF DMAs per norm.
"""

import numpy as np
import ml_dtypes

import concourse.bass as bass
import concourse.mybir as mybir
import concourse.tile as tile
from concourse import bacc
from concourse.bass_utils import run_bass_kernel_spmd

B, N, F, C = 2, 1024, 2048, 64
NB = 5
EPS = 1e-5
NCORES = 8

F32 = mybir.dt.float32
BF16 = mybir.dt.bfloat16
FP8 = mybir.dt.float8e4
NP_BF16 = ml_dtypes.bfloat16
NP_FP8 = ml_dtypes.float8_e4m3
AF = mybir.ActivationFunctionType
ALU = mybir.AluOpType
RG = [list(range(NCORES))]


def _build():
    nc = bacc.Bacc(
        "TRN2",
        target_bir_lowering=False,
        debug=False,
        enable_asserts=False,
        num_devices=NCORES,
    )

    # ---------------- DRAM I/O ----------------
    dit_d = nc.dram_tensor("dit", [128, 32, 2048], FP8, kind="ExternalInput")
    diat_d = nc.dram_tensor("diat", [128, 64, 1024], FP8, kind="ExternalInput")
    inpT_d = nc.dram_tensor("inpT", [3, B, N], BF16, kind="ExternalInput")
    w_in_d = nc.dram_tensor("w_in", [3, C], BF16, kind="ExternalInput")
    b_in2_d = nc.dram_tensor("b_in2", [128, 1], F32, kind="ExternalInput")
    w0_d = nc.dram_tensor("w0", [128, NB, C], FP8, kind="ExternalInput")
    w1_d = nc.dram_tensor("w1", [128, NB, C], FP8, kind="ExternalInput")
    b0_d = nc.dram_tensor("b0", [1, NB, C], BF16, kind="ExternalInput")
    b1_d = nc.dram_tensor("b1", [1, NB, C], BF16, kind="ExternalInput")
    g0_d = nc.dram_tensor("g0", [C, NB], F32, kind="ExternalInput")
    be0_d = nc.dram_tensor("be0", [C, NB], F32, kind="ExternalInput")
    g1_d = nc.dram_tensor("g1", [C, NB], F32, kind="ExternalInput")
    be1_d = nc.dram_tensor("be1", [C, NB], F32, kind="ExternalInput")
    msel_d = nc.dram_tensor("msel", [128, 1], F32, kind="ExternalInput")
    bn2g_d = nc.dram_tensor("bn2g", [C, 1], F32, kind="ExternalInput")
    bn2b_d = nc.dram_tensor("bn2b", [C, 1], F32, kind="ExternalInput")
    w2_d = nc.dram_tensor("w2", [128, C], BF16, kind="ExternalInput")
    b2_d = nc.dram_tensor("b2", [1, C], BF16, kind="ExternalInput")
    maskc_d = nc.dram_tensor("maskc", [128, 8, B], BF16, kind="ExternalInput")
    maskrow_d = nc.dram_tensor("maskrow", [B, N], BF16, kind="ExternalInput")
    wfc_d = nc.dram_tensor("wfc", [C, 10], BF16, kind="ExternalInput")
    bfc_d = nc.dram_tensor("bfc", [B, 10], F32, kind="ExternalInput")
    out_d = nc.dram_tensor("out", [B, 10], F32, kind="ExternalOutput")

    with tile.TileContext(nc) as tc:
        with (
            tc.tile_pool(name="res", bufs=1) as res,
            tc.tile_pool(name="sb", bufs=2) as sb,
            tc.tile_pool(name="sc", bufs=2) as sc,
            tc.tile_pool(name="st", bufs=4) as st,
            tc.tile_pool(name="pacc", bufs=1, space="PSUM") as pacc,
            tc.tile_pool(name="px", bufs=2, space="PSUM") as px,
            tc.tile_pool(name="pm", bufs=1, space="PSUM") as pm,
            tc.tile_pool(name="dram", bufs=2, space="DRAM") as dram,
        ):
            # ------------- resident loads -------------
            dit_sb = res.tile([128, 32, 2048], FP8)
            diat_sb = res.tile([128, 64, 1024], FP8)
            nc.sync.dma_start(dit_sb[:], dit_d.ap())
            nc.sync.dma_start(diat_sb[:], diat_d.ap())

            def load(name, shape, dtype, src):
                t = res.tile(shape, dtype, name=name)
                nc.sync.dma_start(t[:], src.ap())
                return t

            w_in_sb = load("w_in_sb", [3, C], BF16, w_in_d)
            b_in2_sb = load("b_in2_sb", [128, 1], F32, b_in2_d)
            w0_sb = load("w0_sb", [128, NB, C], FP8, w0_d)
            w1_sb = load("w1_sb", [128, NB, C], FP8, w1_d)
            b0_sb = load("b0_sb", [1, NB, C], BF16, b0_d)
            b1_sb = load("b1_sb", [1, NB, C], BF16, b1_d)
            g0_sb = load("g0_sb", [C, NB], F32, g0_d)
            be0_sb = load("be0_sb", [C, NB], F32, be0_d)
            g1_sb = load("g1_sb", [C, NB], F32, g1_d)
            be1_sb = load("be1_sb", [C, NB], F32, be1_d)
            msel_sb = load("msel_sb", [128, 1], F32, msel_d)
            bn2g_sb = load("bn2g_sb", [C, 1], F32, bn2g_d)
            bn2b_sb = load("bn2b_sb", [C, 1], F32, bn2b_d)
            w2_sb = load("w2_sb", [128, C], BF16, w2_d)
            b2_sb = load("b2_sb", [1, C], BF16, b2_d)
            maskc_sb = load("maskc_sb", [128, 8, B], BF16, maskc_d)
            maskrow_sb = load("maskrow_sb", [B, N], BF16, maskrow_d)
            wfc_sb = load("wfc_sb", [C, 10], BF16, wfc_d)
            bfc_sb = load("bfc_sb", [B, 10], F32, bfc_d)
            inpT_sb = load("inpT_sb", [3, B, N], BF16, inpT_d)

            ones_bf = res.tile([1, 128], BF16)
            nc.vector.memset(ones_bf[:], 1.0)

            # ------------- state: both samples stacked on partitions ------
            vTB = res.tile([128, N], BF16)   # partition 64*s + c
            fTB = res.tile([128, F], BF16)

            # initial v = inputs @ W_in + b_in
            for h in range(2):
                psI = pm.tile([128, 512], F32, tag="misc", name=f"psI{h}")
                for s in range(B):
                    nc.tensor.matmul(
                        psI[64 * s:64 * (s + 1), :],
                        w_in_sb[:],
                        inpT_sb[:, s, 512 * h:512 * (h + 1)],
                        start=True, stop=True,
                        tile_position=(0, 64 * s),
                    )
                nc.vector.tensor_scalar(
                    vTB[:, 512 * h:512 * (h + 1)], psI[:],
                    b_in2_sb[:].opt(), None, ALU.add,
                )
            nc.vector.memset(fTB[:], 0.0)

            def elu_stats(src, R, nm):
                """elu(src)->[128,R] bf16 + per-partition (sum, sumsq)."""
                e = sc.tile([128, R], BF16, tag="eT", name=f"e{nm}")
                nc.scalar.activation(e[:], src, AF.Exp)
                nc.vector.tensor_scalar(e[:], e[:], -1.0, 0.0, ALU.add, ALU.min)
                ev = sc.tile([128, R], BF16, tag="evT", name=f"ev{nm}")
                ssum = st.tile([128, 1], F32, tag="ssum", name=f"ssum{nm}")
                nc.vector.scalar_tensor_tensor(
                    ev[:], src, 0.0, e[:], ALU.max, ALU.add, accum_out=ssum[:]
                )
                ssq = st.tile([128, 1], F32, tag="ssq", name=f"ssq{nm}")
                sqd = sc.tile([128, R], BF16, tag="eT", name=f"sq{nm}")
                nc.scalar.activation(sqd[:], ev[:], AF.Square, accum_out=ssq[:])
                return ev, ssum, ssq

            def bn_scales(ssum, ssq, g_ap, be_ap, T, nm, with_sel):
                """-> (scaleM, shnegM) [128,1] f32; affine = x*scaleM - shnegM.

                Cross-partition combines via two tiny SBUF DMAs."""
                stat2 = st.tile([128, 2], F32, tag="stat2", name=f"st2{nm}")
                nc.vector.tensor_copy(stat2[:, 0:1], ssum[:])
                nc.vector.tensor_copy(stat2[:, 1:2], ssq[:])
                statHi = st.tile([C, 2], F32, tag="statHi", name=f"sth{nm}")
                nc.sync.dma_start(statHi[:], stat2[C:128, :])
                cs = st.tile([C, 2], F32, tag="cs", name=f"cs{nm}")
                nc.vector.tensor_add(cs[:], stat2[0:C, :], statHi[:])
                mean = st.tile([C, 1], F32, tag="bns", name=f"mn{nm}")
                nc.vector.tensor_scalar_mul(mean[:], cs[:, 0:1], 1.0 / T)
                m2 = st.tile([C, 1], F32, tag="bns", name=f"m2{nm}")
                nc.vector.tensor_mul(m2[:], mean[:], mean[:])
                varp = st.tile([C, 1], F32, tag="bns", name=f"vp{nm}")
                nc.vector.scalar_tensor_tensor(
                    varp[:], cs[:, 1:2], 1.0 / T, m2[:], ALU.mult, ALU.subtract
                )
                nc.vector.tensor_scalar_add(varp[:], varp[:], EPS)
                sd = st.tile([C, 1], F32, tag="bns", name=f"sd{nm}")
                nc.scalar.activation(sd[:], varp[:], AF.Sqrt)
                rstd = st.tile([C, 1], F32, tag="bns", name=f"rstd{nm}")
                nc.vector.reciprocal(rstd[:], sd[:])
                # scsh rows 0:64 = (scale, shneg); rows 64:128 DMA-duplicated
                scsh = st.tile([128, 2], F32, tag="scsh", name=f"scsh{nm}")
                nc.vector.tensor_mul(scsh[0:C, 0:1], rstd[:], g_ap)
                nc.vector.scalar_tensor_tensor(
                    scsh[0:C, 1:2], mean[:], scsh[0:C, 0:1].opt(), be_ap,
                    ALU.mult, ALU.subtract,
                )
                nc.sync.dma_start(scsh[C:128, :], scsh[0:C, :])
                scaleM = st.tile([128, 1], F32, tag="selsc", name=f"scM{nm}")
                shnegM = st.tile([128, 1], F32, tag="selsh", name=f"shM{nm}")
                if with_sel:
                    nc.vector.tensor_mul(scaleM[:], scsh[:, 0:1], msel_sb[:].opt())
                    nc.vector.tensor_mul(shnegM[:], scsh[:, 1:2], msel_sb[:].opt())
                else:
                    nc.vector.tensor_copy(scaleM[:], scsh[:, 0:1])
                    nc.vector.tensor_copy(shnegM[:], scsh[:, 1:2])
                return scaleM, shnegM

            def make_bias2(b_ap, nm):
                """bias row replicated across 128 partitions via K=1 matmul."""
                psb = px.tile([128, C], F32, tag="psx", name=f"psb{nm}")
                nc.tensor.matmul(psb[:], ones_bf[:], b_ap, start=True, stop=True)
                bias2 = sc.tile([128, C], F32, tag="bias2", name=f"b2_{nm}")
                nc.vector.tensor_copy(bias2[:], psb[:])
                return bias2

            def produce_rows(stuffB, nchunks, w_ap, bias2, out_dtype, nm,
                             lhs_base=None, rhs_base=None):
                """rows[t] [128, C] = elu(stuffB[:,128t:].T @ W + bias)."""
                rows = []
                for t in range(nchunks):
                    psx = px.tile([128, C], F32, tag="psx", name=f"psx{nm}{t}")
                    if lhs_base is None:
                        nc.tensor.matmul(
                            psx[:], stuffB[:, 128 * t:128 * (t + 1)], w_ap,
                            start=True, stop=True,
                        )
                    else:
                        nc.tensor.matmul(
                            psx[:],
                            stuffB[lhs_base:lhs_base + C, 128 * t:128 * (t + 1)],
                            w_ap, start=True, stop=True,
                        )
                    tb = sc.tile([128, C], BF16, tag="tb", name=f"tb{nm}{t}")
                    nc.vector.tensor_tensor(tb[:], psx[:], bias2[:], ALU.add)
                    e2 = sc.tile([128, C], BF16, tag="e2", name=f"e2{nm}{t}")
                    nc.scalar.activation(e2[:], tb[:], AF.Exp)
                    nc.vector.tensor_scalar(e2[:], e2[:], -1.0, 0.0, ALU.add, ALU.min)
                    row = sc.tile([128, C], out_dtype, tag=f"rowt{t}",
                                  name=f"row{nm}{t}")
                    nc.vector.scalar_tensor_tensor(
                        row[:], tb[:], 0.0, e2[:], ALU.max, ALU.add
                    )
                    rows.append(row)
                return rows

            for i in range(NB):
                # ======== v side ========
                ev, ssum, ssq = elu_stats(vTB[:], N, f"v{i}")
                scaleM, shnegM = bn_scales(
                    ssum, ssq, g0_sb[:, i:i + 1].opt(), be0_sb[:, i:i + 1].opt(),
                    float(B * N), f"v{i}", with_sel=True,
                )
                stuffB = sc.tile([128, N], FP8, tag="stuff", name=f"stuffv{i}")
                nc.vector.tensor_scalar(
                    stuffB[:], ev[:], scaleM[:].opt(), shnegM[:].opt(),
                    ALU.mult, ALU.subtract,
                )
                bias2v = make_bias2(b0_sb[:, i, :], f"v{i}")
                xrs = produce_rows(stuffB[:], 8, w0_sb[:, i, :], bias2v,
                                   FP8, f"x{i}")

                # Di matmul, column-tiled over the 4 output groups
                psfB = pacc.tile([128, 512], F32, tag="pf", name=f"pf{i}")
                for kk in range(32):
                    n8, jj = kk // 4, kk % 4
                    for j in range(4):
                        nc.tensor.matmul(
                            psfB[32 * j:32 * j + 16, :],
                            xrs[n8][:, 16 * jj:16 * (jj + 1)],
                            dit_sb[:, kk, 512 * j:512 * (j + 1)],
                            start=(kk == 0), stop=(kk == 31),
                            tile_position=(0, 32 * j),
                        )
                stgf = sb.tile([128, 512], BF16, tag="stgf", name=f"stgf{i}")
                nc.vector.tensor_copy(stgf[:], psfB[:])
                aginf = dram.tile([C, 512], BF16, tag="aginf", name=f"aginf{i}")
                for j in range(4):
                    nc.sync.dma_start(
                        aginf[16 * j:16 * (j + 1), :],
                        stgf[32 * j:32 * j + 16, :],
                    )
                agoutf = dram.tile([NCORES * C, 512], BF16, tag="agoutf",
                                   name=f"agoutf{i}", addr_space="Shared")
                nc.gpsimd.collective_compute(
                    "AllGather", ALU.bypass, replica_groups=RG,
                    ins=[aginf.opt()], outs=[agoutf.opt()],
                )
                agfv = agoutf[:].rearrange("(s r c) n -> s c r n", s=2, r=4)
                gfB = sb.tile([128, 4, 512], BF16, tag="gf", name=f"gf{i}")
                for s in range(B):
                    nc.sync.dma_start(gfB[C * s:C * (s + 1), :, :], agfv[s])
                nc.vector.tensor_tensor(
                    fTB[:], fTB[:], gfB[:].rearrange("c r n -> c (r n)"), ALU.add,
                )

                # ======== f side ========
                ef, fsum, fsq = elu_stats(fTB[:], F, f"f{i}")
                fscaleM, fshnegM = bn_scales(
                    fsum, fsq, g1_sb[:, i:i + 1].opt(), be1_sb[:, i:i + 1].opt(),
                    float(B * F), f"f{i}", with_sel=True,
                )
                stfB = sc.tile([128, F], FP8, tag="stufff", name=f"stufff{i}")
                nc.vector.tensor_scalar(
                    stfB[:], ef[:], fscaleM[:].opt(), fshnegM[:].opt(),
                    ALU.mult, ALU.subtract,
                )
                bias2f = make_bias2(b1_sb[:, i, :], f"f{i}")
                yrs = produce_rows(stfB[:], 16, w1_sb[:, i, :], bias2f,
                                   FP8, f"y{i}")

                psvB = pacc.tile([128, 512], F32, tag="pv", name=f"pv{i}")
                for kk in range(64):
                    pc, jj = kk // 4, kk % 4
                    for nt in range(2):
                        nc.tensor.matmul(
                            psvB[32 * nt:32 * nt + 16, :],
                            yrs[pc][:, 16 * jj:16 * (jj + 1)],
                            diat_sb[:, kk, 512 * nt:512 * (nt + 1)],
                            start=(kk == 0), stop=(kk == 63),
                            tile_position=(0, 32 * nt),
                        )
                stgv = sb.tile([128, 512], BF16, tag="stgv", name=f"stgv{i}")
                nc.vector.tensor_copy(stgv[:], psvB[:])
                aginv = dram.tile([C, 256], BF16, tag="aginv", name=f"aginv{i}")
                for nt in range(2):
                    for jj2 in range(2):
                        nc.sync.dma_start(
                            aginv[16 * (2 * nt + jj2):16 * (2 * nt + jj2 + 1), :],
                            stgv[32 * nt:32 * nt + 16,
                                 256 * jj2:256 * (jj2 + 1)],
                        )
                agoutv = dram.tile([NCORES * C, 256], BF16, tag="agoutv",
                                   name=f"agoutv{i}", addr_space="Shared")
                nc.gpsimd.collective_compute(
                    "AllGather", ALU.bypass, replica_groups=RG,
                    ins=[aginv.opt()], outs=[agoutv.opt()],
                )
                agvv = agoutv[:].rearrange("(s r c) n -> s c r n", s=2, r=4)
                gvB = sb.tile([128, 4, 256], BF16, tag="gv", name=f"gv{i}")
                for s in range(B):
                    nc.sync.dma_start(gvB[C * s:C * (s + 1), :, :], agvv[s])
                nc.vector.tensor_tensor(
                    vTB[:], vTB[:], gvB[:].rearrange("c r n -> c (r n)"), ALU.add,
                )

            # ======== head (both samples, no selection) ========
            hev, hsum, hsq = elu_stats(vTB[:], N, "h")
            hscale, hshneg = bn_scales(
                hsum, hsq, bn2g_sb[:].opt(), bn2b_sb[:].opt(),
                float(B * N), "h", with_sel=False,
            )
            stuff2B = sc.tile([128, N], BF16, tag="stuff", name="stuff2B")
            nc.vector.tensor_scalar(
                stuff2B[:], hev[:], hscale[:].opt(), hshneg[:].opt(),
                ALU.mult, ALU.subtract,
            )
            bias2h = make_bias2(b2_sb[:], "h")
            pooled = sb.tile([C, B], BF16, tag="pooled")
            for s in range(B):
                v2r = produce_rows(
                    stuff2B[:], 8, w2_sb[64 * s:64 * (s + 1), :], bias2h,
                    BF16, f"h{s}", lhs_base=64 * s,
                )
                pp = pm.tile([C, 1], F32, tag="misc", name=f"pp{s}")
                for t in range(8):
                    nc.tensor.matmul(
                        pp[:], v2r[t][:], maskc_sb[:, t, s:s + 1],
                        start=(t == 0), stop=(t == 7),
                    )
                nc.vector.tensor_copy(pooled[:, s:s + 1], pp[:])
            msum = st.tile([B, 1], F32, tag="hd", name="msum")
            nc.vector.tensor_reduce(
                msum[:], maskrow_sb[:], mybir.AxisListType.X, ALU.add
            )
            rec = st.tile([B, 1], F32, tag="hd", name="rec")
            nc.vector.reciprocal(rec[:], msum[:])
            pl = pm.tile([B, 10], F32, tag="misc", name="pl")
            nc.tensor.matmul(pl[:], pooled[:], wfc_sb[:], start=True, stop=True)
            lu = sb.tile([B, 10], F32, tag="hd2", name="lu")
            nc.vector.scalar_tensor_tensor(
                lu[:], pl[:], rec[:].opt(), bfc_sb[:], ALU.mult, ALU.add
            )
            rmax = st.tile([B, 1], F32, tag="hd", name="rmax")
            nc.vector.tensor_reduce(rmax[:], lu[:], mybir.AxisListType.X, ALU.max)
            t2 = sb.tile([B, 10], F32, tag="hd2", name="t2")
            nc.vector.tensor_scalar(t2[:], lu[:], rmax[:].opt(), None, ALU.subtract)
            et = sb.tile([B, 10], F32, tag="hd2", name="et")
            se = st.tile([B, 1], F32, tag="hd", name="se")
            nc.scalar.activation(et[:], t2[:], AF.Exp, accum_out=se[:])
            ls = st.tile([B, 1], F32, tag="hd", name="ls")
            nc.scalar.activation(ls[:], se[:], AF.Ln)
            outv = sb.tile([B, 10], F32, tag="hd2", name="outv")
            nc.vector.tensor_scalar(outv[:], t2[:], ls[:].opt(), None, ALU.subtract)
            nc.sync.dma_start(out_d.ap(), outv[:])

    nc.compile()
    return nc


_NC = None


def _get_nc():
    global _NC
    if _NC is None:
        _NC = _build()
    return _NC


def _host_prep(inputs):
    """Build the 8 per-core input maps. Core c: sample s=c//4, slice r=c%4."""
    Di = np.ascontiguousarray(np.asarray(inputs["Di"]), np.float32)
    DiA = np.ascontiguousarray(np.asarray(inputs["DiA"]), np.float32)
    inp = np.asarray(inputs["inputs"], np.float32)
    mask = np.asarray(inputs["mask"], np.float32)[:, :, 0]   # [2, 1024]

    def dup(a):  # stack weights for both sample halves on K
        return np.concatenate([a, a], axis=0)

    base = {}
    base["w_in"] = np.asarray(inputs["W_in"]).astype(NP_BF16)
    base["b_in2"] = np.tile(
        np.asarray(inputs["b_in"], np.float32).reshape(C, 1), (2, 1))
    base["w0"] = dup(np.ascontiguousarray(
        np.asarray(inputs["rn_W0"]).transpose(1, 0, 2))).astype(NP_FP8)
    base["w1"] = dup(np.ascontiguousarray(
        np.asarray(inputs["rn_W1"]).transpose(1, 0, 2))).astype(NP_FP8)
    base["b0"] = np.asarray(inputs["rn_b0"]).astype(NP_BF16)[None, :, :]
    base["b1"] = np.asarray(inputs["rn_b1"]).astype(NP_BF16)[None, :, :]
    base["g0"] = np.ascontiguousarray(
        np.asarray(inputs["rn_g0"]).T).astype(np.float32)
    base["be0"] = np.ascontiguousarray(
        np.asarray(inputs["rn_be0"]).T).astype(np.float32)
    base["g1"] = np.ascontiguousarray(
        np.asarray(inputs["rn_g1"]).T).astype(np.float32)
    base["be1"] = np.ascontiguousarray(
        np.asarray(inputs["rn_be1"]).T).astype(np.float32)
    base["bn2g"] = np.asarray(inputs["bn2_g"]).astype(np.float32).reshape(C, 1)
    base["bn2b"] = np.asarray(inputs["bn2_b"]).astype(np.float32).reshape(C, 1)
    base["w2"] = dup(np.asarray(inputs["W2"])).astype(NP_BF16)
    base["b2"] = np.asarray(inputs["b2"]).astype(NP_BF16).reshape(1, C)
    base["wfc"] = np.asarray(inputs["Wfc"]).astype(NP_BF16)
    base["bfc"] = np.broadcast_to(
        np.asarray(inputs["bfc"], np.float32), (B, 10)).copy()
    base["inpT"] = np.ascontiguousarray(inp.transpose(2, 0, 1)).astype(NP_BF16)
    base["maskc"] = np.ascontiguousarray(
        mask.reshape(2, 8, 128).transpose(2, 1, 0)).astype(NP_BF16)
    base["maskrow"] = mask.astype(NP_BF16)

    in_maps = []
    for c in range(NCORES):
        s, r = c // 4, c % 4
        m = dict(base)
        Dr = Di[s].reshape(F, 4, N, 4)          # [p, j, n, jj]
        P4 = Dr[512 * r:512 * (r + 1)]          # [512, 4, 1024, 4]
        DiTg = P4.reshape(512, 4, 8, 128, 4).transpose(2, 4, 3, 1, 0) \
                 .reshape(4096, 2048)           # rows (n8,jj,n'), cols (j,p')
        m["dit"] = np.ascontiguousarray(
            DiTg.reshape(32, 128, 2048).transpose(1, 0, 2)).astype(NP_FP8)
        A = DiA[s].reshape(N, 4, F, 4)          # [n, j, p, jj]
        A4 = A[256 * r:256 * (r + 1)]           # [256, 4, 2048, 4]
        DiATg = A4.reshape(256, 4, 16, 128, 4).transpose(2, 4, 3, 1, 0) \
                  .reshape(8192, 1024)          # rows (pc,jj,p''), cols (j,n')
        m["diat"] = np.ascontiguousarray(
            DiATg.reshape(64, 128, 1024).transpose(1, 0, 2)).astype(NP_FP8)
        msel = np.zeros((128, 1), np.float32)
        msel[64 * s:64 * (s + 1)] = 1.0
        m["msel"] = msel
        in_maps.append(m)
    return in_maps


def _run(inputs, trace=False, **kw):
    nc = _get_nc()
    in_maps = _host_prep(inputs)
    res = run_bass_kernel_spmd(
        nc, in_maps, core_ids=list(range(NCORES)), trace=trace, **kw
    )
    out = np.asarray(res.results[0]["out"], np.float32).copy()
    return out, res


def kernel(**inputs):
    out, _ = _run(inputs, trace=False)
    return out


# revision 14
# speedup vs baseline: 1.4528x; 1.0776x over previous
"""Trainium2 Bass kernel for nn_DirModel (quaternion Dirac GNN message passing).

Strategy (8 NeuronCores, B=2 samples):
  - 4 cores per sample: core c owns sample s=c//4 and slice r=c%4 of the face
    rows (Di output) / node rows (DiA output).
  - Di/DiA are host-side transposed/permuted/sliced, cast to fp8(e4m3), and
    kept RESIDENT in SBUF (8+8 MB per core) across all 5 blocks.
  - Big matmuls keep the activations stationary (lhsT [128,16] quaternion
    slices) and stream the resident fp8 matrix as moving operand (N=512),
    accumulating over the contraction in PSUM. The 4 (Di) / 2 (DiA) output
    groups of one contraction chunk run column-tiled (tile_position=(0,32j))
    so their LDWEIGHTS+MATMUL pairs overlap in the PE array.
  - Both samples are stacked on the partition axis: states vTB/fTB are
    [128, rows] bf16 with partitions 64*s+c. All elementwise/BN work runs at
    full 128-lane width; the per-node linears contract K=128 (both samples at
    once) with duplicated weights, the non-owned sample zeroed via a per-core
    selector mask folded into the BN affine (SPMD program stays identical).
  - Per block, slices are exchanged with one 8-rank AllGather per direction.
  - BN batch stats come free: elu(x)=max(x,0)+min(exp(x)-1,0) with sum/sumsq
    piggybacked on accum_out; cross-partition stat combines use two tiny
    SBUF-to-SBUF DMAs per norm.
"""

import numpy as np
import ml_dtypes

import concourse.bass as bass
import concourse.mybir as mybir
import concourse.tile as tile
from concourse import bacc
from concourse.bass_utils import run_bass_kernel_spmd

B, N, F, C = 2, 1024, 2048, 64
NB = 5
EPS = 1e-5
NCORES = 8

F32 = mybir.dt.float32
BF16 = mybir.dt.bfloat16
FP8 = mybir.dt.float8e4
NP_BF16 = ml_dtypes.bfloat16
NP_FP8 = ml_dtypes.float8_e4m3
AF = mybir.ActivationFunctionType
ALU = mybir.AluOpType
RG = [list(range(NCORES))]


def _build():
    nc = bacc.Bacc(
        "TRN2",
        target_bir_lowering=False,
        debug=False,
        enable_asserts=False,
        num_devices=NCORES,
    )

    # ---------------- DRAM I/O ----------------
    dit_d = nc.dram_tensor("dit", [128, 32, 2048], FP8, kind="ExternalInput")
    diat_d = nc.dram_tensor("diat", [128, 64, 1024], FP8, kind="ExternalInput")
    inpT_d = nc.dram_tensor("inpT", [3, B, N], BF16, kind="ExternalInput")
    w_in_d = nc.dram_tensor("w_in", [3, C], BF16, kind="ExternalInput")
    b_in2_d = nc.dram_tensor("b_in2", [128, 1], F32, kind="ExternalInput")
    w0_d = nc.dram_tensor("w0", [128, NB, C], FP8, kind="ExternalInput")
    w1_d = nc.dram_tensor("w1", [128, NB, C], FP8, kind="ExternalInput")
    b0_d = nc.dram_tensor("b0", [1, NB, C], BF16, kind="ExternalInput")
    b1_d = nc.dram_tensor("b1", [1, NB, C], BF16, kind="ExternalInput")
    g0_d = nc.dram_tensor("g0", [C, NB], F32, kind="ExternalInput")
    be0_d = nc.dram_tensor("be0", [C, NB], F32, kind="ExternalInput")
    g1_d = nc.dram_tensor("g1", [C, NB], F32, kind="ExternalInput")
    be1_d = nc.dram_tensor("be1", [C, NB], F32, kind="ExternalInput")
    msel_d = nc.dram_tensor("msel", [128, 1], F32, kind="ExternalInput")
    bn2g_d = nc.dram_tensor("bn2g", [C, 1], F32, kind="ExternalInput")
    bn2b_d = nc.dram_tensor("bn2b", [C, 1], F32, kind="ExternalInput")
    w2_d = nc.dram_tensor("w2", [128, C], BF16, kind="ExternalInput")
    b2_d = nc.dram_tensor("b2", [1, C], BF16, kind="ExternalInput")
    maskc_d = nc.dram_tensor("maskc", [128, 8, B], BF16, kind="ExternalInput")
    maskrow_d = nc.dram_tensor("maskrow", [B, N], BF16, kind="ExternalInput")
    wfc_d = nc.dram_tensor("wfc", [C, 10], BF16, kind="ExternalInput")
    bfc_d = nc.dram_tensor("bfc", [B, 10], F32, kind="ExternalInput")
    out_d = nc.dram_tensor("out", [B, 10], F32, kind="ExternalOutput")

    with tile.TileContext(nc) as tc:
        with (
            tc.tile_pool(name="res", bufs=1) as res,
            tc.tile_pool(name="sb", bufs=2) as sb,
            tc.tile_pool(name="sc", bufs=2) as sc,
            tc.tile_pool(name="st", bufs=4) as st,
            tc.tile_pool(name="pacc", bufs=1, space="PSUM") as pacc,
            tc.tile_pool(name="px", bufs=2, space="PSUM") as px,
            tc.tile_pool(name="pm", bufs=1, space="PSUM") as pm,
            tc.tile_pool(name="dram", bufs=2, space="DRAM") as dram,
        ):
            # ------------- resident loads -------------
            def load(name, shape, dtype, src):
                t = res.tile(shape, dtype, name=name)
                nc.sync.dma_start(t[:], src.ap())
                return t

            w_in_sb = load("w_in_sb", [3, C], BF16, w_in_d)
            b_in2_sb = load("b_in2_sb", [128, 1], F32, b_in2_d)
            w0_sb = load("w0_sb", [128, NB, C], FP8, w0_d)
            w1_sb = load("w1_sb", [128, NB, C], FP8, w1_d)
            b0_sb = load("b0_sb", [1, NB, C], BF16, b0_d)
            b1_sb = load("b1_sb", [1, NB, C], BF16, b1_d)
            g0_sb = load("g0_sb", [C, NB], F32, g0_d)
            be0_sb = load("be0_sb", [C, NB], F32, be0_d)
            g1_sb = load("g1_sb", [C, NB], F32, g1_d)
            be1_sb = load("be1_sb", [C, NB], F32, be1_d)
            msel_sb = load("msel_sb", [128, 1], F32, msel_d)
            bn2g_sb = load("bn2g_sb", [C, 1], F32, bn2g_d)
            bn2b_sb = load("bn2b_sb", [C, 1], F32, bn2b_d)
            w2_sb = load("w2_sb", [128, C], BF16, w2_d)
            b2_sb = load("b2_sb", [1, C], BF16, b2_d)
            maskc_sb = load("maskc_sb", [128, 8, B], BF16, maskc_d)
            maskrow_sb = load("maskrow_sb", [B, N], BF16, maskrow_d)
            wfc_sb = load("wfc_sb", [C, 10], BF16, wfc_d)
            bfc_sb = load("bfc_sb", [B, 10], F32, bfc_d)
            inpT_sb = load("inpT_sb", [3, B, N], BF16, inpT_d)

            # big resident operators AFTER the small loads (same DMA queues)
            # and in 4 chunks each so block-0 compute overlaps the streaming.
            dit_v = dit_d.ap().rearrange("p (a k) n -> p a k n", a=4)
            dit_cs = []
            for a in range(4):
                t = res.tile([128, 8, 2048], FP8, name=f"dit{a}")
                nc.sync.dma_start(t[:], dit_v[:, a])
                dit_cs.append(t)
            diat_v = diat_d.ap().rearrange("p (a k) n -> p a k n", a=4)
            diat_cs = []
            for a in range(4):
                t = res.tile([128, 16, 1024], FP8, name=f"diat{a}")
                nc.sync.dma_start(t[:], diat_v[:, a])
                diat_cs.append(t)

            ones_bf = res.tile([1, 128], BF16)
            nc.vector.memset(ones_bf[:], 1.0)

            # ------------- state: both samples stacked on partitions ------
            vTB = res.tile([128, N], BF16)   # partition 64*s + c
            fTB = res.tile([128, F], BF16)

            # initial v = inputs @ W_in + b_in
            for h in range(2):
                psI = pm.tile([128, 512], F32, tag="misc", name=f"psI{h}")
                for s in range(B):
                    nc.tensor.matmul(
                        psI[64 * s:64 * (s + 1), :],
                        w_in_sb[:],
                        inpT_sb[:, s, 512 * h:512 * (h + 1)],
                        start=True, stop=True,
                        tile_position=(0, 64 * s),
                    )
                nc.vector.tensor_scalar(
                    vTB[:, 512 * h:512 * (h + 1)], psI[:],
                    b_in2_sb[:].opt(), None, ALU.add,
                )
            nc.vector.memset(fTB[:], 0.0)

            def elu_stats(src, R, nm):
                """elu(src)->[128,R] bf16 + per-partition (sum, sumsq)."""
                e = sc.tile([128, R], BF16, tag="eT", name=f"e{nm}")
                nc.scalar.activation(e[:], src, AF.Exp)
                nc.vector.tensor_scalar(e[:], e[:], -1.0, 0.0, ALU.add, ALU.min)
                ev = sc.tile([128, R], BF16, tag="evT", name=f"ev{nm}")
                ssum = st.tile([128, 1], F32, tag="ssum", name=f"ssum{nm}")
                nc.vector.scalar_tensor_tensor(
                    ev[:], src, 0.0, e[:], ALU.max, ALU.add, accum_out=ssum[:]
                )
                ssq = st.tile([128, 1], F32, tag="ssq", name=f"ssq{nm}")
                sqd = sc.tile([128, R], BF16, tag="eT", name=f"sq{nm}")
                nc.scalar.activation(sqd[:], ev[:], AF.Square, accum_out=ssq[:])
                return ev, ssum, ssq

            def bn_scales(ssum, ssq, g_ap, be_ap, T, nm, with_sel):
                """-> (scaleM, shnegM) [128,1] f32; affine = x*scaleM - shnegM.

                Cross-partition combines via two tiny SBUF DMAs."""
                stat2 = st.tile([128, 2], F32, tag="stat2", name=f"st2{nm}")
                nc.vector.tensor_copy(stat2[:, 0:1], ssum[:])
                nc.vector.tensor_copy(stat2[:, 1:2], ssq[:])
                statHi = st.tile([C, 2], F32, tag="statHi", name=f"sth{nm}")
                nc.sync.dma_start(statHi[:], stat2[C:128, :])
                cs = st.tile([C, 2], F32, tag="cs", name=f"cs{nm}")
                nc.vector.tensor_add(cs[:], stat2[0:C, :], statHi[:])
                mean = st.tile([C, 1], F32, tag="bns", name=f"mn{nm}")
                nc.vector.tensor_scalar_mul(mean[:], cs[:, 0:1], 1.0 / T)
                m2 = st.tile([C, 1], F32, tag="bns", name=f"m2{nm}")
                nc.vector.tensor_mul(m2[:], mean[:], mean[:])
                varp = st.tile([C, 1], F32, tag="bns", name=f"vp{nm}")
                nc.vector.scalar_tensor_tensor(
                    varp[:], cs[:, 1:2], 1.0 / T, m2[:], ALU.mult, ALU.subtract
                )
                nc.vector.tensor_scalar_add(varp[:], varp[:], EPS)
                sd = st.tile([C, 1], F32, tag="bns", name=f"sd{nm}")
                nc.scalar.activation(sd[:], varp[:], AF.Sqrt)
                rstd = st.tile([C, 1], F32, tag="bns", name=f"rstd{nm}")
                nc.vector.reciprocal(rstd[:], sd[:])
                # scsh rows 0:64 = (scale, shneg); rows 64:128 DMA-duplicated
                scsh = st.tile([128, 2], F32, tag="scsh", name=f"scsh{nm}")
                nc.vector.tensor_mul(scsh[0:C, 0:1], rstd[:], g_ap)
                nc.vector.scalar_tensor_tensor(
                    scsh[0:C, 1:2], mean[:], scsh[0:C, 0:1].opt(), be_ap,
                    ALU.mult, ALU.subtract,
                )
                nc.sync.dma_start(scsh[C:128, :], scsh[0:C, :])
                scaleM = st.tile([128, 1], F32, tag="selsc", name=f"scM{nm}")
                shnegM = st.tile([128, 1], F32, tag="selsh", name=f"shM{nm}")
                if with_sel:
                    nc.vector.tensor_mul(scaleM[:], scsh[:, 0:1], msel_sb[:].opt())
                    nc.vector.tensor_mul(shnegM[:], scsh[:, 1:2], msel_sb[:].opt())
                else:
                    nc.vector.tensor_copy(scaleM[:], scsh[:, 0:1])
                    nc.vector.tensor_copy(shnegM[:], scsh[:, 1:2])
                return scaleM, shnegM

            def make_bias2(b_ap, nm):
                """bias row replicated across 128 partitions via K=1 matmul."""
                psb = px.tile([128, C], F32, tag="psx", name=f"psb{nm}")
                nc.tensor.matmul(psb[:], ones_bf[:], b_ap, start=True, stop=True)
                bias2 = sc.tile([128, C], F32, tag="bias2", name=f"b2_{nm}")
                nc.vector.tensor_copy(bias2[:], psb[:])
                return bias2

            def produce_rows(stuffB, nchunks, w_ap, bias2, out_dtype, nm,
                             lhs_base=None, rhs_base=None):
                """rows[t] [128, C] = elu(stuffB[:,128t:].T @ W + bias)."""
                rows = []
                for t in range(nchunks):
                    psx = px.tile([128, C], F32, tag="psx", name=f"psx{nm}{t}")
                    if lhs_base is None:
                        nc.tensor.matmul(
                            psx[:], stuffB[:, 128 * t:128 * (t + 1)], w_ap,
                            start=True, stop=True,
                        )
                    else:
                        nc.tensor.matmul(
                            psx[:],
                            stuffB[lhs_base:lhs_base + C, 128 * t:128 * (t + 1)],
                            w_ap, start=True, stop=True,
                        )
                    tb = sc.tile([128, C], BF16, tag="tb", name=f"tb{nm}{t}")
                    nc.vector.tensor_tensor(tb[:], psx[:], bias2[:], ALU.add)
                    e2 = sc.tile([128, C], BF16, tag="e2", name=f"e2{nm}{t}")
                    nc.scalar.activation(e2[:], tb[:], AF.Exp)
                    nc.vector.tensor_scalar(e2[:], e2[:], -1.0, 0.0, ALU.add, ALU.min)
                    row = sc.tile([128, C], out_dtype, tag=f"rowt{t}",
                                  name=f"row{nm}{t}")
                    nc.vector.scalar_tensor_tensor(
                        row[:], tb[:], 0.0, e2[:], ALU.max, ALU.add
                    )
                    rows.append(row)
                return rows

            for i in range(NB):
                # ======== v side ========
                ev, ssum, ssq = elu_stats(vTB[:], N, f"v{i}")
                scaleM, shnegM = bn_scales(
                    ssum, ssq, g0_sb[:, i:i + 1].opt(), be0_sb[:, i:i + 1].opt(),
                    float(B * N), f"v{i}", with_sel=True,
                )
                stuffB = sc.tile([128, N], FP8, tag="stuff", name=f"stuffv{i}")
                nc.vector.tensor_scalar(
                    stuffB[:], ev[:], scaleM[:].opt(), shnegM[:].opt(),
                    ALU.mult, ALU.subtract,
                )
                bias2v = make_bias2(b0_sb[:, i, :], f"v{i}")
                xrs = produce_rows(stuffB[:], 8, w0_sb[:, i, :], bias2v,
                                   FP8, f"x{i}")

                # Di matmul, column-tiled over the 4 output groups
                psfB = pacc.tile([128, 512], F32, tag="pf", name=f"pf{i}")
                for kk in range(32):
                    n8, jj = kk // 4, kk % 4
                    for j in range(4):
                        nc.tensor.matmul(
                            psfB[32 * j:32 * j + 16, :],
                            xrs[n8][:, 16 * jj:16 * (jj + 1)],
                            dit_cs[kk // 8][:, kk % 8, 512 * j:512 * (j + 1)],
                            start=(kk == 0), stop=(kk == 31),
                            tile_position=(0, 32 * j),
                        )
                stgf = sb.tile([128, 512], BF16, tag="stgf", name=f"stgf{i}")
                nc.vector.tensor_copy(stgf[:], psfB[:])
                aginf = dram.tile([C, 512], BF16, tag="aginf", name=f"aginf{i}")
                for j in range(4):
                    nc.sync.dma_start(
                        aginf[16 * j:16 * (j + 1), :],
                        stgf[32 * j:32 * j + 16, :],
                    )
                agoutf = dram.tile([NCORES * C, 512], BF16, tag="agoutf",
                                   name=f"agoutf{i}", addr_space="Shared")
                nc.gpsimd.collective_compute(
                    "AllGather", ALU.bypass, replica_groups=RG,
                    ins=[aginf.opt()], outs=[agoutf.opt()],
                )
                agfv = agoutf[:].rearrange("(s r c) n -> s c r n", s=2, r=4)
                gfB = sb.tile([128, 4, 512], BF16, tag="gf", name=f"gf{i}")
                for s in range(B):
                    nc.sync.dma_start(gfB[C * s:C * (s + 1), :, :], agfv[s])
                nc.vector.tensor_tensor(
                    fTB[:], fTB[:], gfB[:].rearrange("c r n -> c (r n)"), ALU.add,
                )

                # ======== f side ========
                ef, fsum, fsq = elu_stats(fTB[:], F, f"f{i}")
                fscaleM, fshnegM = bn_scales(
                    fsum, fsq, g1_sb[:, i:i + 1].opt(), be1_sb[:, i:i + 1].opt(),
                    float(B * F), f"f{i}", with_sel=True,
                )
                stfB = sc.tile([128, F], FP8, tag="stufff", name=f"stufff{i}")
                nc.vector.tensor_scalar(
                    stfB[:], ef[:], fscaleM[:].opt(), fshnegM[:].opt(),
                    ALU.mult, ALU.subtract,
                )
                bias2f = make_bias2(b1_sb[:, i, :], f"f{i}")
                yrs = produce_rows(stfB[:], 16, w1_sb[:, i, :], bias2f,
                                   FP8, f"y{i}")

                psvB = pacc.tile([128, 256], F32, tag="pv", name=f"pv{i}")
                for kk in range(64):
                    pc, jj = kk // 4, kk % 4
                    for j in range(4):
                        nc.tensor.matmul(
                            psvB[32 * j:32 * j + 16, :],
                            yrs[pc][:, 16 * jj:16 * (jj + 1)],
                            diat_cs[kk // 16][:, kk % 16,
                                              256 * j:256 * (j + 1)],
                            start=(kk == 0), stop=(kk == 63),
                            tile_position=(0, 32 * j),
                        )
                stgv = sb.tile([128, 256], BF16, tag="stgv", name=f"stgv{i}")
                nc.vector.tensor_copy(stgv[:], psvB[:])
                aginv = dram.tile([C, 256], BF16, tag="aginv", name=f"aginv{i}")
                for j in range(4):
                    nc.sync.dma_start(
                        aginv[16 * j:16 * (j + 1), :],
                        stgv[32 * j:32 * j + 16, :],
                    )
                agoutv = dram.tile([NCORES * C, 256], BF16, tag="agoutv",
                                   name=f"agoutv{i}", addr_space="Shared")
                nc.gpsimd.collective_compute(
                    "AllGather", ALU.bypass, replica_groups=RG,
                    ins=[aginv.opt()], outs=[agoutv.opt()],
                )
                agvv = agoutv[:].rearrange("(s r c) n -> s c r n", s=2, r=4)
                gvB = sb.tile([128, 4, 256], BF16, tag="gv", name=f"gv{i}")
                for s in range(B):
                    nc.sync.dma_start(gvB[C * s:C * (s + 1), :, :], agvv[s])
                nc.vector.tensor_tensor(
                    vTB[:], vTB[:], gvB[:].rearrange("c r n -> c (r n)"), ALU.add,
                )

            # ======== head (both samples, no selection) ========
            hev, hsum, hsq = elu_stats(vTB[:], N, "h")
            hscale, hshneg = bn_scales(
                hsum, hsq, bn2g_sb[:].opt(), bn2b_sb[:].opt(),
                float(B * N), "h", with_sel=False,
            )
            stuff2B = sc.tile([128, N], BF16, tag="stuff", name="stuff2B")
            nc.vector.tensor_scalar(
                stuff2B[:], hev[:], hscale[:].opt(), hshneg[:].opt(),
                ALU.mult, ALU.subtract,
            )
            bias2h = make_bias2(b2_sb[:], "h")
            pooled = sb.tile([C, B], BF16, tag="pooled")
            for s in range(B):
                v2r = produce_rows(
                    stuff2B[:], 8, w2_sb[64 * s:64 * (s + 1), :], bias2h,
                    BF16, f"h{s}", lhs_base=64 * s,
                )
                pp = pm.tile([C, 1], F32, tag="misc", name=f"pp{s}")
                for t in range(8):
                    nc.tensor.matmul(
                        pp[:], v2r[t][:], maskc_sb[:, t, s:s + 1],
                        start=(t == 0), stop=(t == 7),
                    )
                nc.vector.tensor_copy(pooled[:, s:s + 1], pp[:])
            msum = st.tile([B, 1], F32, tag="hd", name="msum")
            nc.vector.tensor_reduce(
                msum[:], maskrow_sb[:], mybir.AxisListType.X, ALU.add
            )
            rec = st.tile([B, 1], F32, tag="hd", name="rec")
            nc.vector.reciprocal(rec[:], msum[:])
            pl = pm.tile([B, 10], F32, tag="misc", name="pl")
            nc.tensor.matmul(pl[:], pooled[:], wfc_sb[:], start=True, stop=True)
            lu = sb.tile([B, 10], F32, tag="hd2", name="lu")
            nc.vector.scalar_tensor_tensor(
                lu[:], pl[:], rec[:].opt(), bfc_sb[:], ALU.mult, ALU.add
            )
            rmax = st.tile([B, 1], F32, tag="hd", name="rmax")
            nc.vector.tensor_reduce(rmax[:], lu[:], mybir.AxisListType.X, ALU.max)
            t2 = sb.tile([B, 10], F32, tag="hd2", name="t2")
            nc.vector.tensor_scalar(t2[:], lu[:], rmax[:].opt(), None, ALU.subtract)
            et = sb.tile([B, 10], F32, tag="hd2", name="et")
            se = st.tile([B, 1], F32, tag="hd", name="se")
            nc.scalar.activation(et[:], t2[:], AF.Exp, accum_out=se[:])
            ls = st.tile([B, 1], F32, tag="hd", name="ls")
            nc.scalar.activation(ls[:], se[:], AF.Ln)
            outv = sb.tile([B, 10], F32, tag="hd2", name="outv")
            nc.vector.tensor_scalar(outv[:], t2[:], ls[:].opt(), None, ALU.subtract)
            nc.sync.dma_start(out_d.ap(), outv[:])

    nc.compile()
    return nc


_NC = None


def _get_nc():
    global _NC
    if _NC is None:
        _NC = _build()
    return _NC


def _host_prep(inputs):
    """Build the 8 per-core input maps. Core c: sample s=c//4, slice r=c%4."""
    Di = np.ascontiguousarray(np.asarray(inputs["Di"]), np.float32)
    DiA = np.ascontiguousarray(np.asarray(inputs["DiA"]), np.float32)
    inp = np.asarray(inputs["inputs"], np.float32)
    mask = np.asarray(inputs["mask"], np.float32)[:, :, 0]   # [2, 1024]

    def dup(a):  # stack weights for both sample halves on K
        return np.concatenate([a, a], axis=0)

    base = {}
    base["w_in"] = np.asarray(inputs["W_in"]).astype(NP_BF16)
    base["b_in2"] = np.tile(
        np.asarray(inputs["b_in"], np.float32).reshape(C, 1), (2, 1))
    base["w0"] = dup(np.ascontiguousarray(
        np.asarray(inputs["rn_W0"]).transpose(1, 0, 2))).astype(NP_FP8)
    base["w1"] = dup(np.ascontiguousarray(
        np.asarray(inputs["rn_W1"]).transpose(1, 0, 2))).astype(NP_FP8)
    base["b0"] = np.asarray(inputs["rn_b0"]).astype(NP_BF16)[None, :, :]
    base["b1"] = np.asarray(inputs["rn_b1"]).astype(NP_BF16)[None, :, :]
    base["g0"] = np.ascontiguousarray(
        np.asarray(inputs["rn_g0"]).T).astype(np.float32)
    base["be0"] = np.ascontiguousarray(
        np.asarray(inputs["rn_be0"]).T).astype(np.float32)
    base["g1"] = np.ascontiguousarray(
        np.asarray(inputs["rn_g1"]).T).astype(np.float32)
    base["be1"] = np.ascontiguousarray(
        np.asarray(inputs["rn_be1"]).T).astype(np.float32)
    base["bn2g"] = np.asarray(inputs["bn2_g"]).astype(np.float32).reshape(C, 1)
    base["bn2b"] = np.asarray(inputs["bn2_b"]).astype(np.float32).reshape(C, 1)
    base["w2"] = dup(np.asarray(inputs["W2"])).astype(NP_BF16)
    base["b2"] = np.asarray(inputs["b2"]).astype(NP_BF16).reshape(1, C)
    base["wfc"] = np.asarray(inputs["Wfc"]).astype(NP_BF16)
    base["bfc"] = np.broadcast_to(
        np.asarray(inputs["bfc"], np.float32), (B, 10)).copy()
    base["inpT"] = np.ascontiguousarray(inp.transpose(2, 0, 1)).astype(NP_BF16)
    base["maskc"] = np.ascontiguousarray(
        mask.reshape(2, 8, 128).transpose(2, 1, 0)).astype(NP_BF16)
    base["maskrow"] = mask.astype(NP_BF16)

    in_maps = []
    for c in range(NCORES):
        s, r = c // 4, c % 4
        m = dict(base)
        Dr = Di[s].reshape(F, 4, N, 4)          # [p, j, n, jj]
        P4 = Dr[512 * r:512 * (r + 1)]          # [512, 4, 1024, 4]
        DiTg = P4.reshape(512, 4, 8, 128, 4).transpose(2, 4, 3, 1, 0) \
                 .reshape(4096, 2048)           # rows (n8,jj,n'), cols (j,p')
        m["dit"] = np.ascontiguousarray(
            DiTg.reshape(32, 128, 2048).transpose(1, 0, 2)).astype(NP_FP8)
        A = DiA[s].reshape(N, 4, F, 4)          # [n, j, p, jj]
        A4 = A[256 * r:256 * (r + 1)]           # [256, 4, 2048, 4]
        DiATg = A4.reshape(256, 4, 16, 128, 4).transpose(2, 4, 3, 1, 0) \
                  .reshape(8192, 1024)          # rows (pc,jj,p''), cols (j,n')
        m["diat"] = np.ascontiguousarray(
            DiATg.reshape(64, 128, 1024).transpose(1, 0, 2)).astype(NP_FP8)
        msel = np.zeros((128, 1), np.float32)
        msel[64 * s:64 * (s + 1)] = 1.0
        m["msel"] = msel
        in_maps.append(m)
    return in_maps


def _run(inputs, trace=False, **kw):
    nc = _get_nc()
    in_maps = _host_prep(inputs)
    res = run_bass_kernel_spmd(
        nc, in_maps, core_ids=list(range(NCORES)), trace=trace, **kw
    )
    out = np.asarray(res.results[0]["out"], np.float32).copy()
    return out, res


def kernel(**inputs):
    out, _ = _run(inputs, trace=False)
    return out
